# revision 22
# baseline (speedup 1.0000x reference)
"""Trainium2 Bass kernel for CausalSelfAttentionARMA.

Sharding: batch x head-groups across 8 cores. Core c handles batch b=c//4 and
heads 4*(c%4)..4*(c%4)+3 (2 pairs). Column-parallel qkv/k2 projections,
row-parallel output projection with host-side reduction of partials.

Math restructuring (validated vs reference):
  - AR branch: S^T layout (k on partitions, q on free), exp without max
    subtraction (scores are small), rowsum via ones-augmented V, blockwise
    causal at 256-wide q-blocks.
  - MA branch: linear-attention recurrence. y_ma[t] = qa_t . H_t + strict-tril
    diagonal correction, H_t = sum_{s<t} ka_s (x) e_s, e_s = v_{s+1} - y_ar_s.
    The 1/8 attention scale is folded into kaT and the running-H update; the
    kernel accumulates -y_ma (e' = y_div - v_next) and subtracts at the end.
All matmuls in float32r (full PE rate at moving-N >= 256, ~2e-4 accuracy).
"""

import sys

sys.path.insert(0, "/opt/trn_rl_repo")

import math

import numpy as np

import concourse.bass as bass
import concourse.mybir as mybir
import concourse.tile as tile
from concourse import bacc
from concourse.bass_utils import run_bass_kernel_spmd

F32 = mybir.dt.float32
F32R = mybir.dt.float32r
AF = mybir.ActivationFunctionType
ALU = mybir.AluOpType

B, T, D = 2, 2048, 1024
NH, HD = 16, 64
HPC = 4  # heads per core
NCORES = 8
DEBUG = False
TT = T // 128  # 16 t-tiles
QB = T // 256  # 8 q-blocks / superblocks
SCALE = 1.0 / math.sqrt(HD)  # 0.125


def _build():
    nc = bacc.Bacc("TRN2", target_bir_lowering=False, debug=False, num_devices=NCORES)

    xT_d = nc.dram_tensor("xT", [D, T], F32R, kind="ExternalInput").ap()
    xv_d = nc.dram_tensor("xv", [T, 256], F32R, kind="ExternalInput").ap()
    va_d = nc.dram_tensor("va", [T, 260], F32R, kind="ExternalInput").ap()
    wqkT_d = nc.dram_tensor("wqkT", [D, 512], F32R, kind="ExternalInput").ap()
    wk2T_d = nc.dram_tensor("wk2T", [D, 256], F32R, kind="ExternalInput").ap()
    wpT_d = nc.dram_tensor("wpT", [256, D], F32R, kind="ExternalInput").ap()
    bqk_d = nc.dram_tensor("bqk", [128, 4], F32, kind="ExternalInput").ap()
    bk2_d = nc.dram_tensor("bk2", [1, 256], F32R, kind="ExternalInput").ap()
    onesr_d = nc.dram_tensor("onesr", [1, 128], F32R, kind="ExternalInput").ap()
    id2_d = nc.dram_tensor("id2", [128, 64], F32R, kind="ExternalInput").ap()
    id128_d = nc.dram_tensor("id128", [128, 128], F32R, kind="ExternalInput").ap()
    mar_d = nc.dram_tensor("maskAR", [128, 512], F32, kind="ExternalInput").ap()
    mma_d = nc.dram_tensor("maskMA", [128, 512], F32, kind="ExternalInput").ap()
    zrow_d = nc.dram_tensor("zrow", [1, 256], F32R, kind="ExternalInput").ap()

    out_d = nc.dram_tensor("outp", [T, D], F32, kind="ExternalOutput").ap()
    dbg = {}
    if DEBUG:
        for nm, shp in [
            ("qT0", [128, 2048]),
            ("yar0", [128, 2048]),
            ("ee", [T, 256]),
            ("yfin0", [128, 2048]),
        ]:
            dbg[nm] = nc.dram_tensor(nm, shp, F32, kind="ExternalOutput").ap()

    with tile.TileContext(nc) as tc:
        with (
            tc.tile_pool(name="pbig", bufs=8) as pbig,
            tc.tile_pool(name="pper", bufs=1) as pper,
            tc.tile_pool(name="pw2", bufs=8) as pw2,
            tc.tile_pool(name="pw1", bufs=8) as pw1,
            tc.tile_pool(name="pka", bufs=16) as pka,
            tc.tile_pool(name="pe", bufs=4) as pe_pool,
            tc.tile_pool(name="pcst", bufs=1) as pcst,
            tc.tile_pool(name="pPT", bufs=6) as pPT,
            tc.tile_pool(name="pout", bufs=2) as pout,
            tc.tile_pool(name="psA", bufs=2, space="PSUM") as psA,
            tc.tile_pool(name="psB", bufs=4, space="PSUM") as psB,
        ):
            # ---- constants / weights ----
            bqk_t = pcst.tile([128, 4], F32)
            bk2_t = pcst.tile([1, 256], F32R)
            onesr_t = pcst.tile([1, 128], F32R)
            id2_t = pcst.tile([128, 64], F32R)
            id128_t = pcst.tile([128, 128], F32R)
            mar_t = pcst.tile([128, 512], F32)
            mma_t = pcst.tile([128, 512], F32)
            nc.sync.dma_start(out=bqk_t, in_=bqk_d)
            nc.sync.dma_start(out=bk2_t, in_=bk2_d)
            nc.sync.dma_start(out=onesr_t, in_=onesr_d)
            nc.sync.dma_start(out=id2_t, in_=id2_d)
            nc.sync.dma_start(out=id128_t, in_=id128_d)
            nc.sync.dma_start(out=mar_t, in_=mar_d)
            nc.sync.dma_start(out=mma_t, in_=mma_d)

            wpT_t = [
                pcst.tile([128, 1024], F32R, name=f"wpT{p}", tag=f"wpT{p}")
                for p in range(2)
            ]
            for p in range(2):
                nc.sync.dma_start(out=wpT_t[p], in_=wpT_d[p * 128 : (p + 1) * 128, :])

            xT_t = []
            wqk_t = []
            wk2_t = []
            for dc in range(8):
                xt = pbig.tile([128, 2048], F32R, name=f"xT{dc}", tag="big")
                nc.sync.dma_start(out=xt, in_=xT_d[dc * 128 : (dc + 1) * 128, :])
                xT_t.append(xt)
                wq = pw2.tile([128, 520], F32R, name=f"wqk{dc}", tag="w2")
                nc.sync.dma_start(
                    out=wq[:, 0:512], in_=wqkT_d[dc * 128 : (dc + 1) * 128, :]
                )
                wqk_t.append(wq)
                wk = pw1.tile([128, 256], F32R, name=f"wk2{dc}", tag="w1")
                nc.sync.dma_start(out=wk, in_=wk2T_d[dc * 128 : (dc + 1) * 128, :])
                wk2_t.append(wk)

            # ---- phase 1: projections ----
            qT_t = [
                pper.tile([128, 2048], F32R, name=f"qT{p}", tag=f"qT{p}")
                for p in range(2)
            ]
            kT_t = [
                pper.tile([128, 2048], F32R, name=f"kT{p}", tag=f"kT{p}")
                for p in range(2)
            ]
            yT_t = [
                pper.tile([128, 2048], F32R, name=f"yT{p}", tag=f"yT{p}")
                for p in range(2)
            ]

            for p in range(2):
                for sel in range(2):  # 0=q, 1=k
                    tgt = qT_t[p] if sel == 0 else kT_t[p]
                    bcol = sel * 2 + p
                    for tb in range(4):
                        pj = psA.tile([128, 512], F32, tag="A", name="pj")
                        for dc in range(8):
                            nc.tensor.matmul(
                                pj[:],
                                wqk_t[dc][:, sel * 256 + p * 128 : sel * 256 + (p + 1) * 128],
                                xT_t[dc][:, tb * 512 : (tb + 1) * 512],
                                start=(dc == 0),
                                stop=(dc == 7),
                            )
                        nc.scalar.activation(
                            out=tgt[:, tb * 512 : (tb + 1) * 512],
                            in_=pj[:],
                            func=AF.Identity,
                            bias=bqk_t[:, bcol : bcol + 1],
                            scale=1.0,
                        )

            ka_t = []
            for tt in range(TT):
                k2 = psB.tile([128, 256], F32, tag="B", name="k2")
                for dc in range(8):
                    nc.tensor.matmul(
                        k2[:],
                        xT_t[dc][:, tt * 128 : (tt + 1) * 128],
                        wk2_t[dc][:],
                        start=(dc == 0),
                        stop=False,
                    )
                nc.tensor.matmul(k2[:], onesr_t[:], bk2_t[:], start=False, stop=True)
                ka = pka.tile([128, 256], F32R, name=f"ka{tt}", tag="ka")
                nc.scalar.activation(
                    out=ka[:], in_=k2[:], func=AF.Sigmoid, scale=SCALE * 0.02
                )
                ka_t.append(ka)

            # qa = min(q, 0.02*q)  (scale folded downstream)
            qaT_t = []
            for p in range(2):
                qa = pbig.tile([128, 2048], F32R, name=f"qaT{p}", tag="big")
                nc.vector.scalar_tensor_tensor(
                    out=qa[:],
                    in0=qT_t[p][:],
                    scalar=0.02,
                    in1=qT_t[p][:],
                    op0=ALU.mult,
                    op1=ALU.min,
                )
                qaT_t.append(qa)

            # kaT = transpose(ka) * scale
            kaT_t = [
                pbig.tile([128, 2048], F32R, name=f"kaT{p}", tag="big")
                for p in range(2)
            ]
            for tt in range(TT):
                tps = []
                for hh in range(HPC):
                    tp = psB.tile([64, 128], F32R, tag="B", name="tp")
                    nc.tensor.transpose(
                        tp[:], ka_t[tt][:, hh * 64 : hh * 64 + 64], id128_t[:]
                    )
                    tps.append(tp)
                for hh in range(HPC):
                    p, hb = hh // 2, (hh % 2) * 64
                    nc.vector.tensor_scalar_mul(
                        kaT_t[p][hb : hb + 64, tt * 128 : (tt + 1) * 128],
                        tps[hh][:],
                        SCALE,
                    )

            # va tiles (each covers two 128-row t-tiles)
            va_t = []
            for i in range(8):
                va = pw2.tile([128, 520], F32R, name=f"va{i}", tag="w2")
                nc.sync.dma_start(
                    out=va[:, 0:260], in_=va_d[2 * i * 128 : (2 * i + 1) * 128, :]
                )
                nc.sync.dma_start(
                    out=va[:, 260:520],
                    in_=va_d[(2 * i + 1) * 128 : (2 * i + 2) * 128, :],
                )
                va_t.append(va)

            H_run = pw1.tile([128, 128], F32R, tag="Hrun", bufs=1, name="H_run")

            if DEBUG:
                nc.sync.dma_start(out=dbg["qT0"], in_=qT_t[0][:].bitcast(F32))

            # ---- phase 2+3: AR attention and MA recurrence, interleaved ----
            for qb in range(QB):
                J = qb
                # --- AR for all 4 heads at this q-block ---
                nkt = 2 * (qb + 1)
                for p in range(2):
                    pvs = [
                        psB.tile([65, 256], F32, tag="B", name=f"pv{a}")
                        for a in range(2)
                    ]
                    for bs in range(0, nkt, 4):
                        bn = min(4, nkt - bs)
                        # interleave the two heads of the pair for row-group
                        # concurrency on the PE
                        sps_l = [
                            psA.tile([128, 1024], F32, tag="A", name=f"sps{a}")
                            for a in range(2)
                        ]
                        for i in range(bn):
                            kt = bs + i
                            for a in range(2):
                                hb = a * 64
                                nc.tensor.matmul(
                                    sps_l[a][:, i * 256 : (i + 1) * 256],
                                    kT_t[p][hb : hb + 64, kt * 128 : (kt + 1) * 128],
                                    qT_t[p][hb : hb + 64, qb * 256 : (qb + 1) * 256],
                                    start=True,
                                    stop=True,
                                )
                        for a in range(2):
                            hh = 2 * p + a
                            pT = pPT.tile([128, 1024], F32R, tag="PT", name="pT")
                            nc.scalar.activation(
                                out=pT[:, 0 : bn * 256],
                                in_=sps_l[a][:, 0 : bn * 256],
                                func=AF.Exp,
                                scale=SCALE,
                            )
                            if bs + bn == nkt:  # diagonal pair in this batch
                                nc.vector.tensor_mul(
                                    pT[:, (bn - 2) * 256 : bn * 256],
                                    pT[:, (bn - 2) * 256 : bn * 256],
                                    mar_t[:],
                                )
                            for i in range(bn):
                                kt = bs + i
                                nc.tensor.matmul(
                                    pvs[a][:],
                                    va_t[kt // 2][
                                        :,
                                        (kt % 2) * 260
                                        + hh * 65 : (kt % 2) * 260
                                        + hh * 65
                                        + 65,
                                    ],
                                    pT[:, i * 256 : (i + 1) * 256],
                                    start=(kt == 0),
                                    stop=(kt == nkt - 1),
                                )
                    for a in range(2):
                        hh = 2 * p + a
                        hb = a * 64
                        pv = pvs[a]
                        rsr = pw1.tile([1, 256], F32R, tag="w1", name="rsr")
                        with nc.allow_low_precision(reason="f32r recip"):
                            nc.vector.reciprocal(rsr[:], pv[64:65, :])
                        rb = psB.tile([64, 256], F32, tag="B", name="rb")
                        nc.tensor.matmul(
                            rb[:], onesr_t[:, 0:64], rsr[:], start=True, stop=True
                        )
                        nc.scalar.copy(
                            yT_t[p][hb : hb + 64, qb * 256 : (qb + 1) * 256],
                            pv[0:64, :],
                        )
                        nc.vector.tensor_mul(
                            yT_t[p][hb : hb + 64, qb * 256 : (qb + 1) * 256],
                            yT_t[p][hb : hb + 64, qb * 256 : (qb + 1) * 256],
                            rb[:],
                        )

                if DEBUG and qb == QB - 1:
                    nc.sync.dma_start(out=dbg["yar0"], in_=yT_t[0][:].bitcast(F32))

                # --- MA at superblock J = qb ---
                e_t = []
                for z in range(2):
                    kt = 2 * J + z
                    vs = pw1.tile([128, 256], F32R, tag="w1", name="vs")
                    if kt < TT - 1:
                        nc.sync.dma_start(
                            out=vs[:], in_=xv_d[kt * 128 + 1 : (kt + 1) * 128 + 1, :]
                        )
                    else:
                        nc.sync.dma_start(
                            out=vs[0:127, :], in_=xv_d[kt * 128 + 1 : (kt + 1) * 128, :]
                        )
                        nc.sync.dma_start(out=vs[127:128, :], in_=zrow_d[:])
                    et = pe_pool.tile([128, 256], F32R, tag="e", name="et")
                    for hh in range(HPC):
                        p, hb = hh // 2, (hh % 2) * 64
                        ytp = psB.tile([128, 64], F32R, tag="B", name="ytp")
                        nc.tensor.transpose(
                            ytp[:],
                            yT_t[p][hb : hb + 64, kt * 128 : (kt + 1) * 128],
                            id2_t[hb : hb + 64, :],
                        )
                        nc.vector.tensor_sub(
                            et[:, hh * 64 : hh * 64 + 64], ytp[:],
                            vs[:, hh * 64 : hh * 64 + 64],
                        )
                    e_t.append(et)
                    if DEBUG:
                        nc.sync.dma_start(
                            out=dbg["ee"][kt * 128 : (kt + 1) * 128, :],
                            in_=et[:].bitcast(F32),
                        )

                ymas = []
                for hh in range(HPC):
                    p, hb = hh // 2, (hh % 2) * 64
                    yma = psB.tile([64, 256], F32, tag="B", name="yma")
                    ymas.append(yma)
                    if J > 0:
                        nc.tensor.matmul(
                            yma[:],
                            H_run[hb : hb + 64, p * 64 : p * 64 + 64],
                            qaT_t[p][hb : hb + 64, J * 256 : (J + 1) * 256],
                            start=True,
                            stop=False,
                        )
                for p in range(2):
                    sd_l = [
                        psA.tile([128, 512], F32, tag="A", name=f"sd{a}")
                        for a in range(2)
                    ]
                    for z in range(2):
                        kt = 2 * J + z
                        for a in range(2):
                            hb = a * 64
                            nc.tensor.matmul(
                                sd_l[a][:, z * 256 : (z + 1) * 256],
                                kaT_t[p][hb : hb + 64, kt * 128 : (kt + 1) * 128],
                                qaT_t[p][hb : hb + 64, J * 256 : (J + 1) * 256],
                                start=True,
                                stop=True,
                            )
                    for a in range(2):
                        hh = 2 * p + a
                        sdm = pPT.tile([128, 512], F32R, tag="PT", name="sdm")
                        nc.vector.tensor_mul(sdm[:], sd_l[a][:], mma_t[:])
                        for z in range(2):
                            nc.tensor.matmul(
                                ymas[hh][:],
                                e_t[z][:, hh * 64 : hh * 64 + 64],
                                sdm[:, z * 256 : (z + 1) * 256],
                                start=(J == 0 and z == 0),
                                stop=(z == 1),
                            )
                for hh in range(HPC):
                    p, hb = hh // 2, (hh % 2) * 64
                    nc.vector.tensor_sub(
                        yT_t[p][hb : hb + 64, J * 256 : (J + 1) * 256],
                        yT_t[p][hb : hb + 64, J * 256 : (J + 1) * 256],
                        ymas[hh][:],
                    )
                if J < QB - 1:
                    H_ps = psB.tile([64, 256], F32, tag="B", name="H_ps")
                    for hh in range(HPC):
                        for z in range(2):
                            nc.tensor.matmul(
                                H_ps[0:64, hh * 64 : hh * 64 + 64],
                                ka_t[2 * J + z][:, hh * 64 : hh * 64 + 64],
                                e_t[z][:, hh * 64 : hh * 64 + 64],
                                start=(z == 0),
                                stop=(z == 1),
                            )
                    for hh in range(HPC):
                        p, hb = hh // 2, (hh % 2) * 64
                        if J == 0:
                            nc.vector.tensor_scalar_mul(
                                H_run[hb : hb + 64, p * 64 : p * 64 + 64],
                                H_ps[0:64, hh * 64 : hh * 64 + 64],
                                SCALE,
                            )
                        else:
                            nc.vector.scalar_tensor_tensor(
                                out=H_run[hb : hb + 64, p * 64 : p * 64 + 64],
                                in0=H_ps[0:64, hh * 64 : hh * 64 + 64],
                                scalar=SCALE,
                                in1=H_run[hb : hb + 64, p * 64 : p * 64 + 64],
                                op0=ALU.mult,
                                op1=ALU.add,
                            )

            if DEBUG:
                nc.sync.dma_start(out=dbg["yfin0"], in_=yT_t[0][:].bitcast(F32))

            # ---- phase 4: output projection ----
            for tt in range(TT):
                op = psA.tile([128, 1024], F32, tag="A", name="op")
                for p in range(2):
                    for nb in range(2):
                        nc.tensor.matmul(
                            op[:, nb * 512 : (nb + 1) * 512],
                            yT_t[p][:, tt * 128 : (tt + 1) * 128],
                            wpT_t[p][:, nb * 512 : (nb + 1) * 512],
                            start=(p == 0),
                            stop=(p == 1),
                        )
                ob = pout.tile([128, 1024], F32, tag="ob", name="ob")
                if tt % 2 == 0:
                    nc.scalar.copy(ob[:], op[:])
                else:
                    nc.vector.tensor_copy(ob[:], op[:])
                nc.sync.dma_start(out=out_d[tt * 128 : (tt + 1) * 128, :], in_=ob[:])

    nc.compile()
    return nc


_NC_CACHE = None


def _get_nc():
    global _NC_CACHE
    if _NC_CACHE is None:
        _NC_CACHE = _build()
    return _NC_CACHE


def _prep_in_maps(x, w_attn, b_attn, w_k2, b_k2, w_proj, b_proj):
    x = np.asarray(x, np.float32)
    w_attn = np.asarray(w_attn, np.float32)
    b_attn = np.asarray(b_attn, np.float32)
    w_k2 = np.asarray(w_k2, np.float32)
    b_k2 = np.asarray(b_k2, np.float32)
    w_proj = np.asarray(w_proj, np.float32)

    l = np.arange(128)[:, None]
    c = np.arange(256)[None, :]
    mar = np.concatenate(
        [(l <= c).astype(np.float32), (128 + l <= c).astype(np.float32)], axis=1
    )
    mma = np.concatenate(
        [(l < c).astype(np.float32), (128 + l < c).astype(np.float32)], axis=1
    )
    id2 = np.concatenate([np.eye(64, dtype=np.float32)] * 2, axis=0)
    id128 = np.eye(128, dtype=np.float32)
    onesr = np.ones((1, 128), np.float32)
    zrow = np.zeros((1, 256), np.float32)

    in_maps = []
    for cidx in range(NCORES):
        b = cidx // 4
        g = cidx % 4
        hcols = slice(g * 256, (g + 1) * 256)
        xb = x[b]  # (T, D)
        xv = np.ascontiguousarray(xb[:, hcols])  # (T, 256)
        va = np.empty((T, 260), np.float32)
        for hh in range(HPC):
            va[:, hh * 65 : hh * 65 + 64] = xv[:, hh * 64 : (hh + 1) * 64]
            va[:, hh * 65 + 64] = 1.0
        wq = w_attn[g * 256 : (g + 1) * 256, :]  # (256, D)
        wk = w_attn[D + g * 256 : D + (g + 1) * 256, :]
        wqkT = np.concatenate([wq.T, wk.T], axis=1)  # (D, 512)
        wk2T = np.ascontiguousarray(w_k2[g * 256 : (g + 1) * 256, :].T)  # (D, 256)
        wpT = np.ascontiguousarray(w_proj[:, hcols].T)  # (256, D)
        bqk = np.stack(
            [
                b_attn[g * 256 : g * 256 + 128],
                b_attn[g * 256 + 128 : g * 256 + 256],
                b_attn[D + g * 256 : D + g * 256 + 128],
                b_attn[D + g * 256 + 128 : D + g * 256 + 256],
            ],
            axis=1,
        ).astype(np.float32)  # (128, 4)
        bk2 = b_k2[g * 256 : (g + 1) * 256].reshape(1, 256).astype(np.float32)

        in_maps.append(
            {
                "xT": np.ascontiguousarray(xb.T),
                "xv": xv,
                "va": va,
                "wqkT": np.ascontiguousarray(wqkT),
                "wk2T": wk2T,
                "wpT": wpT,
                "bqk": bqk,
                "bk2": bk2,
                "onesr": onesr,
                "id2": id2,
                "id128": id128,
                "maskAR": mar,
                "maskMA": mma,
                "zrow": zrow,
            }
        )
    return in_maps


def _run(inputs, trace=False, runs=2):
    in_maps = _prep_in_maps(**inputs)
    nc = _get_nc()
    res = None
    # run twice: guards against first-execution cold-state flakes
    for _ in range(max(1, runs)):
        res = run_bass_kernel_spmd(
            nc, in_maps, core_ids=list(range(NCORES)), trace=trace
        )
    b_proj = np.asarray(inputs["b_proj"], np.float32)
    out = np.zeros((B, T, D), np.float32)
    for cidx in range(NCORES):
        out[cidx // 4] += res.results[cidx]["outp"]
    out += 2.0 * b_proj
    return out, res


def kernel(**inputs) -> np.ndarray:
    out, _ = _run(inputs, trace=False)
    return out


# revision 23
# speedup vs baseline: 1.0057x; 1.0057x over previous
"""Trainium2 Bass kernel for CausalSelfAttentionARMA.

Sharding: batch x head-groups across 8 cores. Core c handles batch b=c//4 and
heads 4*(c%4)..4*(c%4)+3 (2 pairs). Column-parallel qkv/k2 projections,
row-parallel output projection with host-side reduction of partials.

Math restructuring (validated vs reference):
  - AR branch: S^T layout (k on partitions, q on free), exp without max
    subtraction (scores are small), rowsum via ones-augmented V, blockwise
    causal at 256-wide q-blocks.
  - MA branch: linear-attention recurrence. y_ma[t] = qa_t . H_t + strict-tril
    diagonal correction, H_t = sum_{s<t} ka_s (x) e_s, e_s = v_{s+1} - y_ar_s.
    The 1/8 attention scale is folded into kaT and the running-H update; the
    kernel accumulates -y_ma (e' = y_div - v_next) and subtracts at the end.
All matmuls in float32r (full PE rate at moving-N >= 256, ~2e-4 accuracy).
"""

import sys

sys.path.insert(0, "/opt/trn_rl_repo")

import math

import numpy as np

import concourse.bass as bass
import concourse.mybir as mybir
import concourse.tile as tile
from concourse import bacc
from concourse.bass_utils import run_bass_kernel_spmd

F32 = mybir.dt.float32
F32R = mybir.dt.float32r
AF = mybir.ActivationFunctionType
ALU = mybir.AluOpType

B, T, D = 2, 2048, 1024
NH, HD = 16, 64
HPC = 4  # heads per core
NCORES = 8
DEBUG = False
TT = T // 128  # 16 t-tiles
QB = T // 256  # 8 q-blocks / superblocks
SCALE = 1.0 / math.sqrt(HD)  # 0.125


def _build():
    nc = bacc.Bacc("TRN2", target_bir_lowering=False, debug=False, num_devices=NCORES)

    xT_d = nc.dram_tensor("xT", [D, T], F32R, kind="ExternalInput").ap()
    xv_d = nc.dram_tensor("xv", [T, 256], F32R, kind="ExternalInput").ap()
    va_d = nc.dram_tensor("va", [T, 260], F32R, kind="ExternalInput").ap()
    wqkT_d = nc.dram_tensor("wqkT", [D, 512], F32R, kind="ExternalInput").ap()
    wk2T_d = nc.dram_tensor("wk2T", [D, 256], F32R, kind="ExternalInput").ap()
    wpT_d = nc.dram_tensor("wpT", [256, D], F32R, kind="ExternalInput").ap()
    bqk_d = nc.dram_tensor("bqk", [128, 4], F32, kind="ExternalInput").ap()
    bk2_d = nc.dram_tensor("bk2", [1, 256], F32R, kind="ExternalInput").ap()
    onesr_d = nc.dram_tensor("onesr", [1, 128], F32R, kind="ExternalInput").ap()
    id2_d = nc.dram_tensor("id2", [128, 64], F32R, kind="ExternalInput").ap()
    id128_d = nc.dram_tensor("id128", [128, 128], F32R, kind="ExternalInput").ap()
    mar_d = nc.dram_tensor("maskAR", [128, 512], F32, kind="ExternalInput").ap()
    mma_d = nc.dram_tensor("maskMA", [128, 512], F32, kind="ExternalInput").ap()
    zrow_d = nc.dram_tensor("zrow", [1, 256], F32R, kind="ExternalInput").ap()

    out_d = nc.dram_tensor("outp", [T, D], F32, kind="ExternalOutput").ap()
    dbg = {}
    if DEBUG:
        for nm, shp in [
            ("qT0", [128, 2048]),
            ("yar0", [128, 2048]),
            ("ee", [T, 256]),
            ("yfin0", [128, 2048]),
        ]:
            dbg[nm] = nc.dram_tensor(nm, shp, F32, kind="ExternalOutput").ap()

    with tile.TileContext(nc) as tc:
        with (
            tc.tile_pool(name="pbig", bufs=8) as pbig,
            tc.tile_pool(name="pper", bufs=1) as pper,
            tc.tile_pool(name="pw2", bufs=8) as pw2,
            tc.tile_pool(name="pw1", bufs=8) as pw1,
            tc.tile_pool(name="pka", bufs=16) as pka,
            tc.tile_pool(name="pe", bufs=4) as pe_pool,
            tc.tile_pool(name="pcst", bufs=1) as pcst,
            tc.tile_pool(name="pPT", bufs=6) as pPT,
            tc.tile_pool(name="pout", bufs=2) as pout,
            tc.tile_pool(name="psA", bufs=2, space="PSUM") as psA,
            tc.tile_pool(name="psB", bufs=4, space="PSUM") as psB,
        ):
            # ---- constants / weights ----
            bqk_t = pcst.tile([128, 4], F32)
            bk2_t = pcst.tile([1, 256], F32R)
            onesr_t = pcst.tile([1, 128], F32R)
            id2_t = pcst.tile([128, 64], F32R)
            id128_t = pcst.tile([128, 128], F32R)
            mar_t = pcst.tile([128, 512], F32)
            mma_t = pcst.tile([128, 512], F32)
            nc.sync.dma_start(out=bqk_t, in_=bqk_d)
            nc.sync.dma_start(out=bk2_t, in_=bk2_d)
            nc.sync.dma_start(out=onesr_t, in_=onesr_d)
            nc.sync.dma_start(out=id2_t, in_=id2_d)
            nc.sync.dma_start(out=id128_t, in_=id128_d)
            nc.sync.dma_start(out=mar_t, in_=mar_d)
            nc.sync.dma_start(out=mma_t, in_=mma_d)

            wpT_t = [
                pcst.tile([128, 1024], F32R, name=f"wpT{p}", tag=f"wpT{p}")
                for p in range(2)
            ]
            for p in range(2):
                nc.sync.dma_start(out=wpT_t[p], in_=wpT_d[p * 128 : (p + 1) * 128, :])

            xT_t = []
            wqk_t = []
            wk2_t = []
            for dc in range(8):
                xt = pbig.tile([128, 2048], F32R, name=f"xT{dc}", tag="big")
                nc.sync.dma_start(out=xt, in_=xT_d[dc * 128 : (dc + 1) * 128, :])
                xT_t.append(xt)
                wq = pw2.tile([128, 520], F32R, name=f"wqk{dc}", tag="w2")
                nc.sync.dma_start(
                    out=wq[:, 0:512], in_=wqkT_d[dc * 128 : (dc + 1) * 128, :]
                )
                wqk_t.append(wq)
                wk = pw1.tile([128, 256], F32R, name=f"wk2{dc}", tag="w1")
                nc.sync.dma_start(out=wk, in_=wk2T_d[dc * 128 : (dc + 1) * 128, :])
                wk2_t.append(wk)

            # ---- phase 1: projections ----
            qT_t = [
                pper.tile([128, 2048], F32R, name=f"qT{p}", tag=f"qT{p}")
                for p in range(2)
            ]
            kT_t = [
                pper.tile([128, 2048], F32R, name=f"kT{p}", tag=f"kT{p}")
                for p in range(2)
            ]
            yT_t = [
                pper.tile([128, 2048], F32R, name=f"yT{p}", tag=f"yT{p}")
                for p in range(2)
            ]

            for p in range(2):
                for sel in range(2):  # 0=q, 1=k
                    tgt = qT_t[p] if sel == 0 else kT_t[p]
                    bcol = sel * 2 + p
                    for tb in range(4):
                        pj = psA.tile([128, 512], F32, tag="A", name="pj")
                        for dc in range(8):
                            nc.tensor.matmul(
                                pj[:],
                                wqk_t[dc][:, sel * 256 + p * 128 : sel * 256 + (p + 1) * 128],
                                xT_t[dc][:, tb * 512 : (tb + 1) * 512],
                                start=(dc == 0),
                                stop=(dc == 7),
                            )
                        nc.scalar.activation(
                            out=tgt[:, tb * 512 : (tb + 1) * 512],
                            in_=pj[:],
                            func=AF.Identity,
                            bias=bqk_t[:, bcol : bcol + 1],
                            scale=1.0,
                        )

            ka_t = []
            for tt in range(TT):
                k2 = psB.tile([128, 256], F32, tag="B", name="k2")
                for dc in range(8):
                    nc.tensor.matmul(
                        k2[:],
                        xT_t[dc][:, tt * 128 : (tt + 1) * 128],
                        wk2_t[dc][:],
                        start=(dc == 0),
                        stop=False,
                    )
                nc.tensor.matmul(k2[:], onesr_t[:], bk2_t[:], start=False, stop=True)
                ka = pka.tile([128, 256], F32R, name=f"ka{tt}", tag="ka")
                nc.scalar.activation(
                    out=ka[:], in_=k2[:], func=AF.Sigmoid, scale=SCALE * 0.02
                )
                ka_t.append(ka)

            # qa = min(q, 0.02*q)  (scale folded downstream)
            qaT_t = []
            for p in range(2):
                qa = pbig.tile([128, 2048], F32R, name=f"qaT{p}", tag="big")
                nc.vector.scalar_tensor_tensor(
                    out=qa[:],
                    in0=qT_t[p][:],
                    scalar=0.02,
                    in1=qT_t[p][:],
                    op0=ALU.mult,
                    op1=ALU.min,
                )
                qaT_t.append(qa)

            # kaT = transpose(ka) * scale
            kaT_t = [
                pbig.tile([128, 2048], F32R, name=f"kaT{p}", tag="big")
                for p in range(2)
            ]
            for tt in range(TT):
                tps = []
                for hh in range(HPC):
                    tp = psB.tile([64, 128], F32R, tag="B", name="tp")
                    nc.tensor.transpose(
                        tp[:], ka_t[tt][:, hh * 64 : hh * 64 + 64], id128_t[:]
                    )
                    tps.append(tp)
                for hh in range(HPC):
                    p, hb = hh // 2, (hh % 2) * 64
                    nc.vector.tensor_scalar_mul(
                        kaT_t[p][hb : hb + 64, tt * 128 : (tt + 1) * 128],
                        tps[hh][:],
                        SCALE,
                    )

            # va tiles (each covers two 128-row t-tiles)
            va_t = []
            for i in range(8):
                va = pw2.tile([128, 520], F32R, name=f"va{i}", tag="w2")
                nc.sync.dma_start(
                    out=va[:, 0:260], in_=va_d[2 * i * 128 : (2 * i + 1) * 128, :]
                )
                nc.sync.dma_start(
                    out=va[:, 260:520],
                    in_=va_d[(2 * i + 1) * 128 : (2 * i + 2) * 128, :],
                )
                va_t.append(va)

            H_run = pw1.tile([128, 128], F32R, tag="Hrun", bufs=1, name="H_run")

            if DEBUG:
                nc.sync.dma_start(out=dbg["qT0"], in_=qT_t[0][:].bitcast(F32))

            # ---- phase 2+3: AR attention and MA recurrence, interleaved ----
            for qb in range(QB):
                J = qb
                # --- AR for all 4 heads at this q-block ---
                nkt = 2 * (qb + 1)
                for p in range(2):
                    pvs = [
                        psB.tile([65, 256], F32, tag="B", name=f"pv{a}")
                        for a in range(2)
                    ]
                    for bs in range(0, nkt, 4):
                        bn = min(4, nkt - bs)
                        # interleave the two heads of the pair for row-group
                        # concurrency on the PE
                        sps_l = [
                            psA.tile([128, 1024], F32, tag="A", name=f"sps{a}")
                            for a in range(2)
                        ]
                        for i in range(bn):
                            kt = bs + i
                            for a in range(2):
                                hb = a * 64
                                nc.tensor.matmul(
                                    sps_l[a][:, i * 256 : (i + 1) * 256],
                                    kT_t[p][hb : hb + 64, kt * 128 : (kt + 1) * 128],
                                    qT_t[p][hb : hb + 64, qb * 256 : (qb + 1) * 256],
                                    start=True,
                                    stop=True,
                                )
                        for a in range(2):
                            hh = 2 * p + a
                            pT = pPT.tile([128, 1024], F32R, tag="PT", name="pT")
                            nc.scalar.activation(
                                out=pT[:, 0 : bn * 256],
                                in_=sps_l[a][:, 0 : bn * 256],
                                func=AF.Exp,
                                scale=SCALE,
                            )
                            if bs + bn == nkt:  # diagonal pair in this batch
                                nc.vector.tensor_mul(
                                    pT[:, (bn - 2) * 256 : bn * 256],
                                    pT[:, (bn - 2) * 256 : bn * 256],
                                    mar_t[:],
                                )
                            for i in range(bn):
                                kt = bs + i
                                nc.tensor.matmul(
                                    pvs[a][:],
                                    va_t[kt // 2][
                                        :,
                                        (kt % 2) * 260
                                        + hh * 65 : (kt % 2) * 260
                                        + hh * 65
                                        + 65,
                                    ],
                                    pT[:, i * 256 : (i + 1) * 256],
                                    start=(kt == 0),
                                    stop=(kt == nkt - 1),
                                )
                    for a in range(2):
                        hh = 2 * p + a
                        hb = a * 64
                        pv = pvs[a]
                        rsr = pw1.tile([1, 256], F32R, tag="w1", name="rsr")
                        with nc.allow_low_precision(reason="f32r recip"):
                            nc.vector.reciprocal(rsr[:], pv[64:65, :])
                        rb = psB.tile([64, 256], F32, tag="B", name="rb")
                        nc.tensor.matmul(
                            rb[:], onesr_t[:, 0:64], rsr[:], start=True, stop=True
                        )
                        nc.scalar.copy(
                            yT_t[p][hb : hb + 64, qb * 256 : (qb + 1) * 256],
                            pv[0:64, :],
                        )
                        nc.vector.tensor_mul(
                            yT_t[p][hb : hb + 64, qb * 256 : (qb + 1) * 256],
                            yT_t[p][hb : hb + 64, qb * 256 : (qb + 1) * 256],
                            rb[:],
                        )

                if DEBUG and qb == QB - 1:
                    nc.sync.dma_start(out=dbg["yar0"], in_=yT_t[0][:].bitcast(F32))

                # --- MA at superblock J = qb ---
                e_t = []
                for z in range(2):
                    kt = 2 * J + z
                    vs = pw1.tile([128, 256], F32R, tag="w1", name="vs")
                    if kt < TT - 1:
                        nc.sync.dma_start(
                            out=vs[:], in_=xv_d[kt * 128 + 1 : (kt + 1) * 128 + 1, :]
                        )
                    else:
                        nc.sync.dma_start(
                            out=vs[0:127, :], in_=xv_d[kt * 128 + 1 : (kt + 1) * 128, :]
                        )
                        nc.sync.dma_start(out=vs[127:128, :], in_=zrow_d[:])
                    et = pe_pool.tile([128, 256], F32R, tag="e", name="et")
                    for hh in range(HPC):
                        p, hb = hh // 2, (hh % 2) * 64
                        ytp = psB.tile([128, 64], F32R, tag="B", name="ytp")
                        nc.tensor.transpose(
                            ytp[:],
                            yT_t[p][hb : hb + 64, kt * 128 : (kt + 1) * 128],
                            id2_t[hb : hb + 64, :],
                        )
                        nc.vector.tensor_sub(
                            et[:, hh * 64 : hh * 64 + 64], ytp[:],
                            vs[:, hh * 64 : hh * 64 + 64],
                        )
                    e_t.append(et)
                    if DEBUG:
                        nc.sync.dma_start(
                            out=dbg["ee"][kt * 128 : (kt + 1) * 128, :],
                            in_=et[:].bitcast(F32),
                        )

                ymas = []
                for hh in range(HPC):
                    p, hb = hh // 2, (hh % 2) * 64
                    yma = psB.tile([64, 256], F32, tag="B", name="yma")
                    ymas.append(yma)
                    if J > 0:
                        nc.tensor.matmul(
                            yma[:],
                            H_run[hb : hb + 64, p * 64 : p * 64 + 64],
                            qaT_t[p][hb : hb + 64, J * 256 : (J + 1) * 256],
                            start=True,
                            stop=False,
                        )
                for p in range(2):
                    sd_l = [
                        psA.tile([128, 512], F32, tag="A", name=f"sd{a}")
                        for a in range(2)
                    ]
                    for z in range(2):
                        kt = 2 * J + z
                        for a in range(2):
                            hb = a * 64
                            nc.tensor.matmul(
                                sd_l[a][:, z * 256 : (z + 1) * 256],
                                kaT_t[p][hb : hb + 64, kt * 128 : (kt + 1) * 128],
                                qaT_t[p][hb : hb + 64, J * 256 : (J + 1) * 256],
                                start=True,
                                stop=True,
                            )
                    for a in range(2):
                        hh = 2 * p + a
                        sdm = pPT.tile([128, 512], F32R, tag="PT", name="sdm")
                        nc.vector.tensor_mul(sdm[:], sd_l[a][:], mma_t[:])
                        for z in range(2):
                            nc.tensor.matmul(
                                ymas[hh][:],
                                e_t[z][:, hh * 64 : hh * 64 + 64],
                                sdm[:, z * 256 : (z + 1) * 256],
                                start=(J == 0 and z == 0),
                                stop=(z == 1),
                            )
                for hh in range(HPC):
                    p, hb = hh // 2, (hh % 2) * 64
                    nc.vector.tensor_sub(
                        yT_t[p][hb : hb + 64, J * 256 : (J + 1) * 256],
                        yT_t[p][hb : hb + 64, J * 256 : (J + 1) * 256],
                        ymas[hh][:],
                    )
                if J < QB - 1:
                    H_ps = psB.tile([64, 256], F32, tag="B", name="H_ps")
                    for hh in range(HPC):
                        for z in range(2):
                            nc.tensor.matmul(
                                H_ps[0:64, hh * 64 : hh * 64 + 64],
                                ka_t[2 * J + z][:, hh * 64 : hh * 64 + 64],
                                e_t[z][:, hh * 64 : hh * 64 + 64],
                                start=(z == 0),
                                stop=(z == 1),
                            )
                    for hh in range(HPC):
                        p, hb = hh // 2, (hh % 2) * 64
                        if J == 0:
                            nc.vector.tensor_scalar_mul(
                                H_run[hb : hb + 64, p * 64 : p * 64 + 64],
                                H_ps[0:64, hh * 64 : hh * 64 + 64],
                                SCALE,
                            )
                        else:
                            nc.vector.scalar_tensor_tensor(
                                out=H_run[hb : hb + 64, p * 64 : p * 64 + 64],
                                in0=H_ps[0:64, hh * 64 : hh * 64 + 64],
                                scalar=SCALE,
                                in1=H_run[hb : hb + 64, p * 64 : p * 64 + 64],
                                op0=ALU.mult,
                                op1=ALU.add,
                            )

                # ---- output projection for the two finished t-tiles ----
                for tt in (2 * qb, 2 * qb + 1):
                    op = psA.tile([128, 1024], F32, tag="A", name="op")
                    for p in range(2):
                        for nb in range(2):
                            nc.tensor.matmul(
                                op[:, nb * 512 : (nb + 1) * 512],
                                yT_t[p][:, tt * 128 : (tt + 1) * 128],
                                wpT_t[p][:, nb * 512 : (nb + 1) * 512],
                                start=(p == 0),
                                stop=(p == 1),
                            )
                    ob = pout.tile([128, 1024], F32, tag="ob", name="ob")
                    if tt % 2 == 0:
                        nc.scalar.copy(ob[:], op[:])
                    else:
                        nc.vector.tensor_copy(ob[:], op[:])
                    nc.sync.dma_start(
                        out=out_d[tt * 128 : (tt + 1) * 128, :], in_=ob[:]
                    )

            if DEBUG:
                nc.sync.dma_start(out=dbg["yfin0"], in_=yT_t[0][:].bitcast(F32))


    nc.compile()
    return nc


_NC_CACHE = None


def _get_nc():
    global _NC_CACHE
    if _NC_CACHE is None:
        _NC_CACHE = _build()
    return _NC_CACHE


def _prep_in_maps(x, w_attn, b_attn, w_k2, b_k2, w_proj, b_proj):
    x = np.asarray(x, np.float32)
    w_attn = np.asarray(w_attn, np.float32)
    b_attn = np.asarray(b_attn, np.float32)
    w_k2 = np.asarray(w_k2, np.float32)
    b_k2 = np.asarray(b_k2, np.float32)
    w_proj = np.asarray(w_proj, np.float32)

    l = np.arange(128)[:, None]
    c = np.arange(256)[None, :]
    mar = np.concatenate(
        [(l <= c).astype(np.float32), (128 + l <= c).astype(np.float32)], axis=1
    )
    mma = np.concatenate(
        [(l < c).astype(np.float32), (128 + l < c).astype(np.float32)], axis=1
    )
    id2 = np.concatenate([np.eye(64, dtype=np.float32)] * 2, axis=0)
    id128 = np.eye(128, dtype=np.float32)
    onesr = np.ones((1, 128), np.float32)
    zrow = np.zeros((1, 256), np.float32)

    in_maps = []
    for cidx in range(NCORES):
        b = cidx // 4
        g = cidx % 4
        hcols = slice(g * 256, (g + 1) * 256)
        xb = x[b]  # (T, D)
        xv = np.ascontiguousarray(xb[:, hcols])  # (T, 256)
        va = np.empty((T, 260), np.float32)
        for hh in range(HPC):
            va[:, hh * 65 : hh * 65 + 64] = xv[:, hh * 64 : (hh + 1) * 64]
            va[:, hh * 65 + 64] = 1.0
        wq = w_attn[g * 256 : (g + 1) * 256, :]  # (256, D)
        wk = w_attn[D + g * 256 : D + (g + 1) * 256, :]
        wqkT = np.concatenate([wq.T, wk.T], axis=1)  # (D, 512)
        wk2T = np.ascontiguousarray(w_k2[g * 256 : (g + 1) * 256, :].T)  # (D, 256)
        wpT = np.ascontiguousarray(w_proj[:, hcols].T)  # (256, D)
        bqk = np.stack(
            [
                b_attn[g * 256 : g * 256 + 128],
                b_attn[g * 256 + 128 : g * 256 + 256],
                b_attn[D + g * 256 : D + g * 256 + 128],
                b_attn[D + g * 256 + 128 : D + g * 256 + 256],
            ],
            axis=1,
        ).astype(np.float32)  # (128, 4)
        bk2 = b_k2[g * 256 : (g + 1) * 256].reshape(1, 256).astype(np.float32)

        in_maps.append(
            {
                "xT": np.ascontiguousarray(xb.T),
                "xv": xv,
                "va": va,
                "wqkT": np.ascontiguousarray(wqkT),
                "wk2T": wk2T,
                "wpT": wpT,
                "bqk": bqk,
                "bk2": bk2,
                "onesr": onesr,
                "id2": id2,
                "id128": id128,
                "maskAR": mar,
                "maskMA": mma,
                "zrow": zrow,
            }
        )
    return in_maps


def _run(inputs, trace=False, runs=2):
    in_maps = _prep_in_maps(**inputs)
    nc = _get_nc()
    res = None
    # run twice: guards against first-execution cold-state flakes
    for _ in range(max(1, runs)):
        res = run_bass_kernel_spmd(
            nc, in_maps, core_ids=list(range(NCORES)), trace=trace
        )
    b_proj = np.asarray(inputs["b_proj"], np.float32)
    out = np.zeros((B, T, D), np.float32)
    for cidx in range(NCORES):
        out[cidx // 4] += res.results[cidx]["outp"]
    out += 2.0 * b_proj
    return out, res


def kernel(**inputs) -> np.ndarray:
    out, _ = _run(inputs, trace=False)
    return out


# revision 30
# speedup vs baseline: 1.0649x; 1.0589x over previous
"""Trainium2 Bass kernel for CausalSelfAttentionARMA.

Sharding: batch x head-groups across 8 cores. Core c handles batch b=c//4 and
heads 4*(c%4)..4*(c%4)+3 (2 pairs). Column-parallel qkv/k2 projections,
row-parallel output projection with host-side reduction of partials.

Math restructuring (validated vs reference):
  - AR branch: S^T layout (k on partitions, q on free), exp without max
    subtraction (scores are small), rowsum via ones-augmented V, blockwise
    causal at 256-wide q-blocks.
  - MA branch: linear-attention recurrence. y_ma[t] = qa_t . H_t + strict-tril
    diagonal correction, H_t = sum_{s<t} ka_s (x) e_s, e_s = v_{s+1} - y_ar_s.
    The 1/8 attention scale is folded into kaT and the running-H update; the
    kernel accumulates -y_ma (e' = y_div - v_next) and subtracts at the end.
All matmuls in float32r (full PE rate at moving-N >= 256, ~2e-4 accuracy).
"""

import sys

sys.path.insert(0, "/opt/trn_rl_repo")

import math

import numpy as np

import concourse.bass as bass
import concourse.mybir as mybir
import concourse.tile as tile
from concourse import bacc
from concourse.bass_utils import run_bass_kernel_spmd

F32 = mybir.dt.float32
F32R = mybir.dt.float32r
AF = mybir.ActivationFunctionType
ALU = mybir.AluOpType

B, T, D = 2, 2048, 1024
NH, HD = 16, 64
HPC = 4  # heads per core
NCORES = 8
DEBUG = False
PSA_BUFS = 2
PSB_BUFS = 4
PPT_BUFS = 6
TT = T // 128  # 16 t-tiles
QB = T // 256  # 8 q-blocks / superblocks
SCALE = 1.0 / math.sqrt(HD)  # 0.125


def _build():
    nc = bacc.Bacc("TRN2", target_bir_lowering=False, debug=False, num_devices=NCORES)

    xT_d = nc.dram_tensor("xT", [D, T], F32R, kind="ExternalInput").ap()
    xv_d = nc.dram_tensor("xv", [T, 256], F32R, kind="ExternalInput").ap()
    va_d = nc.dram_tensor("va", [T, 260], F32R, kind="ExternalInput").ap()
    wqkT_d = nc.dram_tensor("wqkT", [D, 512], F32R, kind="ExternalInput").ap()
    wk2T_d = nc.dram_tensor("wk2T", [D, 256], F32R, kind="ExternalInput").ap()
    wpT_d = nc.dram_tensor("wpT", [256, D], F32R, kind="ExternalInput").ap()
    bqk_d = nc.dram_tensor("bqk", [128, 4], F32, kind="ExternalInput").ap()
    bk2_d = nc.dram_tensor("bk2", [1, 256], F32R, kind="ExternalInput").ap()
    onesr_d = nc.dram_tensor("onesr", [1, 128], F32R, kind="ExternalInput").ap()
    id2_d = nc.dram_tensor("id2", [128, 64], F32R, kind="ExternalInput").ap()
    id128_d = nc.dram_tensor("id128", [128, 128], F32R, kind="ExternalInput").ap()
    mar_d = nc.dram_tensor("maskAR", [128, 512], F32, kind="ExternalInput").ap()
    mma_d = nc.dram_tensor("maskMA", [128, 512], F32, kind="ExternalInput").ap()
    zrow_d = nc.dram_tensor("zrow", [1, 256], F32R, kind="ExternalInput").ap()

    out_d = nc.dram_tensor("outp", [T, D], F32, kind="ExternalOutput").ap()
    dbg = {}
    if DEBUG:
        for nm, shp in [
            ("qT0", [128, 2048]),
            ("yar0", [128, 2048]),
            ("ee", [T, 256]),
            ("yfin0", [128, 2048]),
        ]:
            dbg[nm] = nc.dram_tensor(nm, shp, F32, kind="ExternalOutput").ap()

    with tile.TileContext(nc) as tc:
        with (
            tc.tile_pool(name="pbig", bufs=8) as pbig,
            tc.tile_pool(name="pper", bufs=1) as pper,
            tc.tile_pool(name="pw2", bufs=8) as pw2,
            tc.tile_pool(name="pw1", bufs=8) as pw1,
            tc.tile_pool(name="pka", bufs=16) as pka,
            tc.tile_pool(name="pe", bufs=4) as pe_pool,
            tc.tile_pool(name="pcst", bufs=1) as pcst,
            tc.tile_pool(name="pPT", bufs=PPT_BUFS) as pPT,
            tc.tile_pool(name="pout", bufs=2) as pout,
            tc.tile_pool(name="psA", bufs=PSA_BUFS, space="PSUM") as psA,
            tc.tile_pool(name="psB", bufs=PSB_BUFS, space="PSUM") as psB,
        ):
            # ---- constants / weights ----
            bqk_t = pcst.tile([128, 4], F32)
            bk2_t = pcst.tile([1, 256], F32R)
            onesr_t = pcst.tile([1, 128], F32R)
            id2_t = pcst.tile([128, 64], F32R)
            id128_t = pcst.tile([128, 128], F32R)
            mar_t = pcst.tile([128, 512], F32)
            mma_t = pcst.tile([128, 512], F32)
            nc.sync.dma_start(out=bqk_t, in_=bqk_d)
            nc.sync.dma_start(out=bk2_t, in_=bk2_d)
            nc.sync.dma_start(out=onesr_t, in_=onesr_d)
            nc.sync.dma_start(out=id2_t, in_=id2_d)
            nc.sync.dma_start(out=id128_t, in_=id128_d)
            nc.sync.dma_start(out=mar_t, in_=mar_d)
            nc.sync.dma_start(out=mma_t, in_=mma_d)

            wpT_t = [
                pcst.tile([128, 1024], F32R, name=f"wpT{p}", tag=f"wpT{p}")
                for p in range(2)
            ]

            xT_t = []
            wqk_t = []
            wk2_t = []
            dma_engs = [nc.sync, nc.scalar]
            for dc in range(8):
                xt = pbig.tile([128, 2048], F32R, name=f"xT{dc}", tag="big")
                xT_t.append(xt)
                wq = pw2.tile([128, 520], F32R, name=f"wqk{dc}", tag="w2")
                dma_engs[dc % 2].dma_start(
                    out=wq[:, 0:512], in_=wqkT_d[dc * 128 : (dc + 1) * 128, :]
                )
                wqk_t.append(wq)
                dma_engs[(dc + 1) % 2].dma_start(
                    out=xt[:, 0:512], in_=xT_d[dc * 128 : (dc + 1) * 128, 0:512]
                )
            for dc in range(8):
                wk = pw1.tile([128, 256], F32R, name=f"wk2{dc}", tag="w1")
                dma_engs[(dc + 1) % 2].dma_start(
                    out=wk, in_=wk2T_d[dc * 128 : (dc + 1) * 128, :]
                )
                wk2_t.append(wk)
            for tb in range(1, 4):
                for dc in range(8):
                    dma_engs[(tb + dc) % 2].dma_start(
                        out=xT_t[dc][:, tb * 512 : (tb + 1) * 512],
                        in_=xT_d[dc * 128 : (dc + 1) * 128, tb * 512 : (tb + 1) * 512],
                    )

            # ---- phase 1: projections ----
            qT_t = [
                pper.tile([128, 2048], F32R, name=f"qT{p}", tag=f"qT{p}")
                for p in range(2)
            ]
            kT_t = [
                pper.tile([128, 2048], F32R, name=f"kT{p}", tag=f"kT{p}")
                for p in range(2)
            ]
            yT_t = [
                pper.tile([128, 2048], F32R, name=f"yT{p}", tag=f"yT{p}")
                for p in range(2)
            ]

            for tb in range(4):
                for p in range(2):
                    for sel in range(2):  # 0=q, 1=k
                        tgt = qT_t[p] if sel == 0 else kT_t[p]
                        bcol = sel * 2 + p
                        pj = psA.tile([128, 512], F32, tag="A", name="pj")
                        for dc in range(8):
                            nc.tensor.matmul(
                                pj[:],
                                wqk_t[dc][:, sel * 256 + p * 128 : sel * 256 + (p + 1) * 128],
                                xT_t[dc][:, tb * 512 : (tb + 1) * 512],
                                start=(dc == 0),
                                stop=(dc == 7),
                            )
                        nc.scalar.activation(
                            out=tgt[:, tb * 512 : (tb + 1) * 512],
                            in_=pj[:],
                            func=AF.Identity,
                            bias=bqk_t[:, bcol : bcol + 1],
                            scale=1.0,
                        )

            ka_t = []
            for tt in range(TT):
                k2 = psB.tile([128, 256], F32, tag="B", name="k2")
                for dc in range(8):
                    nc.tensor.matmul(
                        k2[:],
                        xT_t[dc][:, tt * 128 : (tt + 1) * 128],
                        wk2_t[dc][:],
                        start=(dc == 0),
                        stop=False,
                    )
                nc.tensor.matmul(k2[:], onesr_t[:], bk2_t[:], start=False, stop=True)
                ka = pka.tile([128, 256], F32R, name=f"ka{tt}", tag="ka")
                nc.scalar.activation(
                    out=ka[:], in_=k2[:], func=AF.Sigmoid, scale=SCALE * 0.02
                )
                ka_t.append(ka)

            # qa = min(q, 0.02*q)  (scale folded downstream)
            qaT_t = []
            for p in range(2):
                qa = pbig.tile([128, 2048], F32R, name=f"qaT{p}", tag="big")
                nc.vector.scalar_tensor_tensor(
                    out=qa[:],
                    in0=qT_t[p][:],
                    scalar=0.02,
                    in1=qT_t[p][:],
                    op0=ALU.mult,
                    op1=ALU.min,
                )
                qaT_t.append(qa)

            # kaT = transpose(ka) * scale
            kaT_t = [
                pbig.tile([128, 2048], F32R, name=f"kaT{p}", tag="big")
                for p in range(2)
            ]

            for tt in range(TT):
                tps = []
                for hh in range(HPC):
                    tp = psB.tile([64, 128], F32R, tag="B", name="tp")
                    nc.tensor.transpose(
                        tp[:], ka_t[tt][:, hh * 64 : hh * 64 + 64], id128_t[:]
                    )
                    tps.append(tp)
                for hh in range(HPC):
                    p, hb = hh // 2, (hh % 2) * 64
                    # split PSUM->SBUF copies between ACT and DVE
                    if hh % 2 == 0:
                        nc.scalar.mul(
                            kaT_t[p][hb : hb + 64, tt * 128 : (tt + 1) * 128],
                            tps[hh][:],
                            SCALE,
                        )
                    else:
                        nc.vector.tensor_scalar_mul(
                            kaT_t[p][hb : hb + 64, tt * 128 : (tt + 1) * 128],
                            tps[hh][:],
                            SCALE,
                        )

            # va tiles (each covers two 128-row t-tiles)
            va_t = []
            for i in range(8):
                va = pw2.tile([128, 520], F32R, name=f"va{i}", tag="w2")
                nc.sync.dma_start(
                    out=va[:, 0:260], in_=va_d[2 * i * 128 : (2 * i + 1) * 128, :]
                )
                nc.sync.dma_start(
                    out=va[:, 260:520],
                    in_=va_d[(2 * i + 1) * 128 : (2 * i + 2) * 128, :],
                )
                va_t.append(va)

            for p in range(2):
                nc.sync.dma_start(out=wpT_t[p], in_=wpT_d[p * 128 : (p + 1) * 128, :])
            H_run = pw1.tile([128, 128], F32R, tag="Hrun", bufs=1, name="H_run")

            if DEBUG:
                nc.sync.dma_start(out=dbg["qT0"], in_=qT_t[0][:].bitcast(F32))

            # ---- phase 2+3: AR attention and MA recurrence, interleaved ----
            for qb in range(QB):
                J = qb
                # --- AR for all 4 heads at this q-block ---
                nkt = 2 * (qb + 1)
                for p in range(2):
                    pvs = [
                        psB.tile([65, 256], F32, tag="B", name=f"pv{a}")
                        for a in range(2)
                    ]
                    for bs in range(0, nkt, 4):
                        bn = min(4, nkt - bs)
                        for a in range(2):
                            hb = a * 64
                            hh = 2 * p + a
                            sps = psA.tile([128, 1024], F32, tag="A", name="sps")
                            for i in range(bn):
                                kt = bs + i
                                nc.tensor.matmul(
                                    sps[:, i * 256 : (i + 1) * 256],
                                    kT_t[p][hb : hb + 64, kt * 128 : (kt + 1) * 128],
                                    qT_t[p][hb : hb + 64, qb * 256 : (qb + 1) * 256],
                                    start=True,
                                    stop=True,
                                )
                            pT = pPT.tile([128, 1024], F32R, tag="PT", name="pT")
                            nc.scalar.activation(
                                out=pT[:, 0 : bn * 256],
                                in_=sps[:, 0 : bn * 256],
                                func=AF.Exp,
                                scale=SCALE,
                            )
                            if bs + bn == nkt:  # diagonal pair in this batch
                                nc.gpsimd.tensor_mul(
                                    pT[:, (bn - 2) * 256 : bn * 256],
                                    pT[:, (bn - 2) * 256 : bn * 256],
                                    mar_t[:],
                                )
                            for i in range(bn):
                                kt = bs + i
                                nc.tensor.matmul(
                                    pvs[a][:],
                                    va_t[kt // 2][
                                        :,
                                        (kt % 2) * 260
                                        + hh * 65 : (kt % 2) * 260
                                        + hh * 65
                                        + 65,
                                    ],
                                    pT[:, i * 256 : (i + 1) * 256],
                                    start=(kt == 0),
                                    stop=(kt == nkt - 1),
                                )
                    for a in range(2):
                        hh = 2 * p + a
                        hb = a * 64
                        pv = pvs[a]
                        rsr = pw1.tile([1, 256], F32R, tag="w1", name="rsr")
                        with nc.allow_low_precision(reason="f32r recip"):
                            nc.vector.reciprocal(rsr[:], pv[64:65, :])
                        rb = psB.tile([64, 256], F32, tag="B", name="rb")
                        nc.tensor.matmul(
                            rb[:], onesr_t[:, 0:64], rsr[:], start=True, stop=True
                        )
                        nc.scalar.copy(
                            yT_t[p][hb : hb + 64, qb * 256 : (qb + 1) * 256],
                            pv[0:64, :],
                        )
                        nc.vector.tensor_mul(
                            yT_t[p][hb : hb + 64, qb * 256 : (qb + 1) * 256],
                            yT_t[p][hb : hb + 64, qb * 256 : (qb + 1) * 256],
                            rb[:],
                        )

                if DEBUG and qb == QB - 1:
                    nc.sync.dma_start(out=dbg["yar0"], in_=yT_t[0][:].bitcast(F32))

                # diag scores + masks first (independent of e)
                sdms = []
                for p in range(2):
                    sd_l = [
                        psA.tile([128, 512], F32, tag="A", name=f"sd{a}")
                        for a in range(2)
                    ]
                    for z in range(2):
                        kt = 2 * J + z
                        for a in range(2):
                            hb = a * 64
                            nc.tensor.matmul(
                                sd_l[a][:, z * 256 : (z + 1) * 256],
                                kaT_t[p][hb : hb + 64, kt * 128 : (kt + 1) * 128],
                                qaT_t[p][hb : hb + 64, J * 256 : (J + 1) * 256],
                                start=True,
                                stop=True,
                            )
                    for a in range(2):
                        sdm = pPT.tile([128, 512], F32R, tag="PT", name="sdm")
                        nc.vector.tensor_mul(sdm[:], sd_l[a][:], mma_t[:])
                        sdms.append(sdm)

                e_t = []
                for z in range(2):
                    kt = 2 * J + z
                    vs = pw1.tile([128, 256], F32R, tag="w1", name="vs")
                    if kt < TT - 1:
                        nc.sync.dma_start(
                            out=vs[:], in_=xv_d[kt * 128 + 1 : (kt + 1) * 128 + 1, :]
                        )
                    else:
                        nc.sync.dma_start(
                            out=vs[0:127, :], in_=xv_d[kt * 128 + 1 : (kt + 1) * 128, :]
                        )
                        nc.sync.dma_start(out=vs[127:128, :], in_=zrow_d[:])
                    et = pe_pool.tile([128, 256], F32R, tag="e", name="et")
                    for hh in range(HPC):
                        p, hb = hh // 2, (hh % 2) * 64
                        ytp = psB.tile([128, 64], F32R, tag="B", name="ytp")
                        nc.tensor.transpose(
                            ytp[:],
                            yT_t[p][hb : hb + 64, kt * 128 : (kt + 1) * 128],
                            id2_t[hb : hb + 64, :],
                        )
                        nc.vector.tensor_sub(
                            et[:, hh * 64 : hh * 64 + 64], ytp[:],
                            vs[:, hh * 64 : hh * 64 + 64],
                        )
                    e_t.append(et)
                    if DEBUG:
                        nc.sync.dma_start(
                            out=dbg["ee"][kt * 128 : (kt + 1) * 128, :],
                            in_=et[:].bitcast(F32),
                        )

                ymas = []
                for hh in range(HPC):
                    p, hb = hh // 2, (hh % 2) * 64
                    yma = psB.tile([64, 256], F32, tag="B", name="yma")
                    ymas.append(yma)
                    if J > 0:
                        nc.tensor.matmul(
                            yma[:],
                            H_run[hb : hb + 64, p * 64 : p * 64 + 64],
                            qaT_t[p][hb : hb + 64, J * 256 : (J + 1) * 256],
                            start=True,
                            stop=False,
                        )
                    sdm = sdms[hh]
                    for z in range(2):
                        nc.tensor.matmul(
                            yma[:],
                            e_t[z][:, hh * 64 : hh * 64 + 64],
                            sdm[:, z * 256 : (z + 1) * 256],
                            start=(J == 0 and z == 0),
                            stop=(z == 1),
                        )
                for hh in range(HPC):
                    p, hb = hh // 2, (hh % 2) * 64
                    nc.vector.tensor_sub(
                        yT_t[p][hb : hb + 64, J * 256 : (J + 1) * 256],
                        yT_t[p][hb : hb + 64, J * 256 : (J + 1) * 256],
                        ymas[hh][:],
                    )
                if J < QB - 1:
                    H_ps = psB.tile([64, 256], F32, tag="B", name="H_ps")
                    for hh in range(HPC):
                        for z in range(2):
                            nc.tensor.matmul(
                                H_ps[0:64, hh * 64 : hh * 64 + 64],
                                ka_t[2 * J + z][:, hh * 64 : hh * 64 + 64],
                                e_t[z][:, hh * 64 : hh * 64 + 64],
                                start=(z == 0),
                                stop=(z == 1),
                            )
                    for hh in range(HPC):
                        p, hb = hh // 2, (hh % 2) * 64
                        if J == 0:
                            nc.vector.tensor_scalar_mul(
                                H_run[hb : hb + 64, p * 64 : p * 64 + 64],
                                H_ps[0:64, hh * 64 : hh * 64 + 64],
                                SCALE,
                            )
                        else:
                            nc.vector.scalar_tensor_tensor(
                                out=H_run[hb : hb + 64, p * 64 : p * 64 + 64],
                                in0=H_ps[0:64, hh * 64 : hh * 64 + 64],
                                scalar=SCALE,
                                in1=H_run[hb : hb + 64, p * 64 : p * 64 + 64],
                                op0=ALU.mult,
                                op1=ALU.add,
                            )

                # ---- output projection for the two finished t-tiles ----
                for tt in (2 * qb, 2 * qb + 1):
                    op = psA.tile([128, 1024], F32, tag="A", name="op")
                    for p in range(2):
                        for nb in range(2):
                            nc.tensor.matmul(
                                op[:, nb * 512 : (nb + 1) * 512],
                                yT_t[p][:, tt * 128 : (tt + 1) * 128],
                                wpT_t[p][:, nb * 512 : (nb + 1) * 512],
                                start=(p == 0),
                                stop=(p == 1),
                            )
                    ob = pout.tile([128, 1024], F32, tag="ob", name="ob")
                    if tt % 2 == 0:
                        nc.scalar.copy(ob[:], op[:])
                    else:
                        nc.vector.tensor_copy(ob[:], op[:])
                    nc.sync.dma_start(
                        out=out_d[tt * 128 : (tt + 1) * 128, :], in_=ob[:]
                    )

            if DEBUG:
                nc.sync.dma_start(out=dbg["yfin0"], in_=yT_t[0][:].bitcast(F32))


    nc.compile()
    return nc


_NC_CACHE = None


def _get_nc():
    global _NC_CACHE
    if _NC_CACHE is None:
        _NC_CACHE = _build()
    return _NC_CACHE


def _prep_in_maps(x, w_attn, b_attn, w_k2, b_k2, w_proj, b_proj):
    x = np.asarray(x, np.float32)
    w_attn = np.asarray(w_attn, np.float32)
    b_attn = np.asarray(b_attn, np.float32)
    w_k2 = np.asarray(w_k2, np.float32)
    b_k2 = np.asarray(b_k2, np.float32)
    w_proj = np.asarray(w_proj, np.float32)

    l = np.arange(128)[:, None]
    c = np.arange(256)[None, :]
    mar = np.concatenate(
        [(l <= c).astype(np.float32), (128 + l <= c).astype(np.float32)], axis=1
    )
    mma = np.concatenate(
        [(l < c).astype(np.float32), (128 + l < c).astype(np.float32)], axis=1
    )
    id2 = np.concatenate([np.eye(64, dtype=np.float32)] * 2, axis=0)
    id128 = np.eye(128, dtype=np.float32)
    onesr = np.ones((1, 128), np.float32)
    zrow = np.zeros((1, 256), np.float32)

    in_maps = []
    for cidx in range(NCORES):
        b = cidx // 4
        g = cidx % 4
        hcols = slice(g * 256, (g + 1) * 256)
        xb = x[b]  # (T, D)
        xv = np.ascontiguousarray(xb[:, hcols])  # (T, 256)
        va = np.empty((T, 260), np.float32)
        for hh in range(HPC):
            va[:, hh * 65 : hh * 65 + 64] = xv[:, hh * 64 : (hh + 1) * 64]
            va[:, hh * 65 + 64] = 1.0
        wq = w_attn[g * 256 : (g + 1) * 256, :]  # (256, D)
        wk = w_attn[D + g * 256 : D + (g + 1) * 256, :]
        wqkT = np.concatenate([wq.T, wk.T], axis=1)  # (D, 512)
        wk2T = np.ascontiguousarray(w_k2[g * 256 : (g + 1) * 256, :].T)  # (D, 256)
        wpT = np.ascontiguousarray(w_proj[:, hcols].T)  # (256, D)
        bqk = np.stack(
            [
                b_attn[g * 256 : g * 256 + 128],
                b_attn[g * 256 + 128 : g * 256 + 256],
                b_attn[D + g * 256 : D + g * 256 + 128],
                b_attn[D + g * 256 + 128 : D + g * 256 + 256],
            ],
            axis=1,
        ).astype(np.float32)  # (128, 4)
        bk2 = b_k2[g * 256 : (g + 1) * 256].reshape(1, 256).astype(np.float32)

        in_maps.append(
            {
                "xT": np.ascontiguousarray(xb.T),
                "xv": xv,
                "va": va,
                "wqkT": np.ascontiguousarray(wqkT),
                "wk2T": wk2T,
                "wpT": wpT,
                "bqk": bqk,
                "bk2": bk2,
                "onesr": onesr,
                "id2": id2,
                "id128": id128,
                "maskAR": mar,
                "maskMA": mma,
                "zrow": zrow,
            }
        )
    return in_maps


def _run(inputs, trace=False, runs=2):
    in_maps = _prep_in_maps(**inputs)
    nc = _get_nc()
    res = None
    # run twice: guards against first-execution cold-state flakes
    for _ in range(max(1, runs)):
        res = run_bass_kernel_spmd(
            nc, in_maps, core_ids=list(range(NCORES)), trace=trace
        )
    b_proj = np.asarray(inputs["b_proj"], np.float32)
    out = np.zeros((B, T, D), np.float32)
    for cidx in range(NCORES):
        out[cidx // 4] += res.results[cidx]["outp"]
    out += 2.0 * b_proj
    return out, res


def kernel(**inputs) -> np.ndarray:
    out, _ = _run(inputs, trace=False)
    return out
